# revision 3
# baseline (speedup 1.0000x reference)
"""Trainium2 Bass kernel for nn_Decoder (fc + 3-layer GRU + mask).

Strategy: data-parallel over batch B=32 across 8 cores (4 samples/core).
Per core, all compute in gate-major ("ghT") layout: gates/hidden on the
partition dim, (hidden-chunk, batch) in the free dim. The recurrent
matmul keeps w_hh chunks as the stationary operand (bf16 -> FWL weight
load) and the tiny h-vector as the moving operand, accumulating the K
contraction in PSUM. gx (input-gate projections) are precomputed per
16-step chunk as GEMMs from the previous layer's stored h sequence
(layer 0: from the chord embedding stream).
"""

import os
import sys
from contextlib import ExitStack

for _p in ("/opt/trn_rl_repo",):
    if _p not in sys.path:
        sys.path.insert(0, _p)

import numpy as np
import ml_dtypes

import concourse.bass as bass
import concourse.bacc as bacc
import concourse.mybir as mybir
import concourse.tile as tile
from concourse import bass_utils

BF = np.float16
F32 = np.float32
dt = mybir.dt

NCORES = 8
B, T = 32, 512
BS = B // NCORES          # 4 samples per core
TC = 16                   # time-steps per chunk
NCHUNK = T // TC          # 32
HID = 1024                # layer-0 input dim
H = 512                   # GRU hidden
G3 = 3 * H                # 1536
MCH = G3 // 128           # 12 gate-dim chunks (m): order r0..r3 z0..z3 n0..n3
KC = H // 128             # 4 hidden chunks
KC0 = HID // 128          # 8 input chunks for layer 0
FREE = KC * BS            # 16 = (hc, b) free layout used everywhere
REPEAT = 0                # extra timing-only layer-1 passes
ABLATE = ""               # timing experiments: "nogates", "nochain"
RDEV = 0                  # device-side timing repeats of a layer-1 pass

Sigmoid = mybir.ActivationFunctionType.Sigmoid
Tanh = mybir.ActivationFunctionType.Tanh
Relu = mybir.ActivationFunctionType.Relu
Alu = mybir.AluOpType


def _declare_io(nc):
    d = {}
    inp = lambda n, s, t: nc.dram_tensor(n, s, t, kind="ExternalInput").ap()
    d["chordT"] = inp("chordT", [KC0, 128, T, BS], dt.float16)
    d["zT"] = inp("zT", [2, 128, BS], dt.float16)
    d["fcwT"] = inp("fcwT", [2, 128, HID], dt.float16)
    d["fcb"] = inp("fcb", [KC0, 128, 1], dt.float32)
    d["wihT0"] = inp("wihT0", [KC0, 128, G3], dt.float16)
    d["wihT1"] = inp("wihT1", [KC, 128, G3], dt.float16)
    d["wihT2"] = inp("wihT2", [KC, 128, G3], dt.float16)
    d["whhT0"] = inp("whhT0", [KC, 128, G3], dt.float16)
    d["whhT1"] = inp("whhT1", [KC, 128, G3], dt.float16)
    d["whhT2"] = inp("whhT2", [KC, 128, G3], dt.float16)
    d["biasrep"] = inp("biasrep", [3, 128, MCH * BS], dt.float32)
    d["bhhn"] = inp("bhhn", [3, 128, FREE], dt.float32)
    d["iota"] = inp("iota", [128, T], dt.float32)
    d["seqrep"] = inp("seqrep", [128, BS], dt.float32)
    d["outT"] = nc.dram_tensor("outT", [KC, 128, T, BS], dt.float32,
                               kind="ExternalOutput").ap()
    return d


def _build_program(debug=False):
    nc = bacc.Bacc("TRN2", target_bir_lowering=False, debug=debug,
                   num_devices=NCORES)
    io = _declare_io(nc)

    with tile.TileContext(nc) as tc:
        _emit(tc, io)
    nc.compile()
    return nc


def _emit(tc, io):
    nc = tc.nc
    ctx = ExitStack()
    const = ctx.enter_context(tc.tile_pool(name="const", bufs=1))
    stream = ctx.enter_context(tc.tile_pool(name="stream", bufs=3))
    gxp = ctx.enter_context(tc.tile_pool(name="gxp", bufs=2))
    tmp = ctx.enter_context(tc.tile_pool(name="tmp", bufs=3))
    outp = ctx.enter_context(tc.tile_pool(name="outp", bufs=2))
    pgh_pool = ctx.enter_context(tc.tile_pool(name="pgh", bufs=4, space="PSUM"))
    pgx_pool = ctx.enter_context(tc.tile_pool(name="pgx", bufs=2, space="PSUM"))

    # ---- persistent SBUF tensors -------------------------------------
    wih = [const.tile([128, KC0, G3], dt.float16, tag="wih0", name="wih0"),
           const.tile([128, KC, G3], dt.float16, tag="wih1", name="wih1"),
           const.tile([128, KC, G3], dt.float16, tag="wih2", name="wih2")]
    whh = [const.tile([128, KC, G3], dt.float16, tag=f"whh{l}", name=f"whh{l}") for l in range(3)]
    hseq = [const.tile([128, T, FREE], dt.float16, tag=f"hseq{i}", name=f"hseq{i}") for i in range(2)]
    mask = const.tile([128, T, BS], dt.float32, tag="mask")
    gb = [const.tile([128, MCH * BS], dt.float32, tag=f"gb{l}", name=f"gb{l}") for l in range(3)]
    bhhn = const.tile([128, 3, FREE], dt.float32, tag="bhhn")
    fc_hT = const.tile([128, KC0, BS], dt.float16, tag="fchT")
    hfp = const.tile([128, FREE], dt.float32, tag="hfp")
    hbf = const.tile([128, TC, FREE], dt.float16, tag="hbf")

    # ---- load weights / constants ------------------------------------
    for kc in range(KC0):
        nc.sync.dma_start(wih[0][:, kc, :], io["wihT0"][kc])
    for l in (1, 2):
        for kc in range(KC):
            nc.sync.dma_start(wih[l][:, kc, :], io[f"wihT{l}"][kc])
    for l in range(3):
        for kc in range(KC):
            nc.sync.dma_start(whh[l][:, kc, :], io[f"whhT{l}"][kc])
        nc.sync.dma_start(gb[l][:], io["biasrep"][l])
        nc.sync.dma_start(bhhn[:, l, :], io["bhhn"][l])

    # ---- prologue: mask, fc, gB0 -------------------------------------
    with ExitStack() as pctx:
        psb = pctx.enter_context(tc.tile_pool(name="psb", bufs=2))

        iota_sb = psb.tile([128, T], dt.float32, tag="iota")
        seq_sb = psb.tile([128, BS], dt.float32, tag="seq")
        nc.sync.dma_start(iota_sb[:], io["iota"])
        nc.sync.dma_start(seq_sb[:], io["seqrep"])
        for b in range(BS):
            nc.vector.tensor_scalar(mask[:, :, b], iota_sb[:],
                                    seq_sb[:, b:b + 1], None, op0=Alu.is_lt)

        z_sb = psb.tile([128, 2, BS], dt.float16, tag="zsb")
        fcw_sb = psb.tile([128, 2, HID], dt.float16, tag="fcw")
        fcb_sb = psb.tile([128, KC0], dt.float32, tag="fcb")
        for kc in range(2):
            nc.sync.dma_start(z_sb[:, kc, :], io["zT"][kc])
            nc.sync.dma_start(fcw_sb[:, kc, :], io["fcwT"][kc])
        for hc in range(KC0):
            nc.sync.dma_start(fcb_sb[:, hc:hc + 1], io["fcb"][hc])
        for hc in range(KC0):
            pfc = pgx_pool.tile([128, BS], dt.float32, tag="pgx", name="pfc")
            for kc in range(2):
                nc.tensor.matmul(pfc[:], fcw_sb[:, kc, hc * 128:(hc + 1) * 128],
                                 z_sb[:, kc, :], start=(kc == 0), stop=(kc == 1))
            nc.scalar.activation(fc_hT[:, hc, :], pfc[:], Relu,
                                 bias=fcb_sb[:, hc:hc + 1], scale=1.0)
        # gB0 += w_ih0 @ fc_hT
        for m in range(MCH):
            pgb = pgx_pool.tile([128, BS], dt.float32, tag="pgx", name="pgb")
            for kc in range(KC0):
                nc.tensor.matmul(pgb[:], wih[0][:, kc, m * 128:(m + 1) * 128],
                                 fc_hT[:, kc, :], start=(kc == 0),
                                 stop=(kc == KC0 - 1))
            nc.vector.tensor_add(gb[0][:, m * BS:(m + 1) * BS],
                                 gb[0][:, m * BS:(m + 1) * BS], pgb[:])

    # ---- per-layer chunk loop ----------------------------------------
    passes = [0, 1, 2] + [1] * REPEAT + ([1] if RDEV else [])
    for pidx, l in enumerate(passes):
        in_dev_repeat = RDEV and pidx == len(passes) - 1
        rep_cm = tc.For_i(0, RDEV, 1, name="rep") if in_dev_repeat else None
        if rep_cm is not None:
            rep_cm.__enter__()
        kcl = KC0 if l == 0 else KC
        cur = hseq[l % 2]
        prev = hseq[(l - 1) % 2]
        is_last = pidx == len(passes) - 1 if REPEAT == 0 else (pidx == 2)
        nc.gpsimd.memset(hfp[:], 0.0)
        nc.gpsimd.memset(hbf[:, TC - 1, :], 0.0)

        hint = (mybir.EngineType.PE,)
        with tc.For_i(0, T, TC, hint_engines=hint, name=f"pass{pidx}") as i:
            # -- gx GEMM for this chunk --
            if l == 0:
                src = stream.tile([128, TC, KC0, BS], dt.float16, tag="src0")
                for kc in range(KC0):
                    nc.sync.dma_start(src[:, :, kc, :],
                                      io["chordT"][kc, :, bass.ds(i, TC), :])
            else:
                src = stream.tile([128, TC, KC, BS], dt.float16, tag="src12")
                if "nodma" not in ABLATE:
                    nc.sync.dma_start(src[:], prev[:, bass.ds(i, TC), :])
                else:
                    nc.gpsimd.memset(src[:, 0, 0, :], 0.0)

            gx = gxp.tile([128, MCH, TC, BS], dt.float32, tag="gx")
            if "nogemm" in ABLATE:
                nc.gpsimd.memset(gx[:, 0, 0, :], 0.0)
            for m in range(0 if "nogemm" in ABLATE else MCH):
                pgx = pgx_pool.tile([128, TC * BS], dt.float32, tag="pgx")
                for kc in range(kcl):
                    nc.tensor.matmul(
                        pgx[:], wih[l][:, kc, m * 128:(m + 1) * 128],
                        src[:, :, kc, :], start=(kc == 0), stop=(kc == kcl - 1))
                nc.vector.tensor_add(
                    gx[:, m, :, :],
                    pgx[:].rearrange("p (t b) -> p t b", t=TC),
                    gb[l][:, m * BS:(m + 1) * BS]
                    .rearrange("p (o b) -> p o b", o=1).broadcast_to([128, TC, BS]))

            if l == 2 and ABLATE != "nogates":
                mch = stream.tile([128, TC, BS], dt.float32, tag="maskch")
                nc.sync.dma_start(mch[:], mask[:, bass.ds(i, TC), :])
                osb = outp.tile([128, KC, TC, BS], dt.float32, tag="osb")

            # -- TC recurrence steps --
            for s in range(0 if "nosteps" in ABLATE else TC):
                sp = (s - 1) % TC          # previous step's h slot
                if ABLATE == "nochain":
                    sp = TC - 1            # constant rhs: breaks serial chain
                pgh = pgh_pool.tile([128, MCH * BS], dt.float32, tag="pgh")
                for m in range(MCH):
                    for kc in range(KC):
                        nc.tensor.matmul(
                            pgh[:, m * BS:(m + 1) * BS],
                            whh[l][:, kc, m * 128:(m + 1) * 128],
                            hbf[:, sp, kc * BS:(kc + 1) * BS],
                            start=(kc == 0), stop=(kc == KC - 1))
                if ABLATE == "nogates":
                    continue
                arz = tmp.tile([128, 2 * FREE], dt.float32, tag="arz")
                nc.vector.tensor_add(arz[:], pgh[:, 0:2 * FREE],
                                     gx[:, 0:2 * KC, s, :])
                rz = tmp.tile([128, 2 * FREE], dt.float32, tag="rz")
                nc.scalar.activation(rz[:], arz[:], Sigmoid)
                t1 = tmp.tile([128, FREE], dt.float32, tag="t1")
                nc.vector.tensor_add(t1[:], pgh[:, 2 * FREE:3 * FREE],
                                     bhhn[:, l, :])
                rn = tmp.tile([128, FREE], dt.float32, tag="rn")
                nc.vector.tensor_mul(rn[:], t1[:], rz[:, 0:FREE])
                aN = tmp.tile([128, FREE], dt.float32, tag="aN")
                nc.vector.tensor_add(aN[:], rn[:], gx[:, 2 * KC:3 * KC, s, :])
                # zh = z*h_prev and oz = 1-z issue early (before tanh
                # completes) so only 2 ops remain on the post-tanh path:
                # h' = oz*n + zh.
                zh = tmp.tile([128, FREE], dt.float32, tag="zh")
                nc.vector.tensor_mul(zh[:], rz[:, FREE:2 * FREE],
                                     hbf[:, sp, :])
                oz = tmp.tile([128, FREE], dt.float32, tag="oz")
                nc.vector.tensor_scalar(oz[:], rz[:, FREE:2 * FREE],
                                        -1.0, 1.0, op0=Alu.mult, op1=Alu.add)
                n = tmp.tile([128, FREE], dt.float32, tag="n")
                nc.scalar.activation(n[:], aN[:], Tanh)
                u = tmp.tile([128, FREE], dt.float32, tag="u")
                nc.vector.tensor_mul(u[:], oz[:], n[:])
                nc.vector.tensor_add(hbf[:, s, :], u[:], zh[:])
                if l == 2 and ABLATE != "nogates":
                    nc.vector.tensor_mul(
                        osb[:, :, s, :],
                        hbf[:, s, :].rearrange("p (h b) -> p h b", h=KC),
                        mch[:, s:s + 1, :].broadcast_to([128, KC, BS]))

            if l < 2:
                if "nodma" not in ABLATE:
                    nc.sync.dma_start(cur[:, bass.ds(i, TC), :], hbf[:])
            elif ABLATE != "nogates":
                for hc in range(KC):
                    nc.sync.dma_start(io["outT"][hc, :, bass.ds(i, TC), :],
                                      osb[:, hc, :, :])
        if rep_cm is not None:
            rep_cm.__exit__(None, None, None)
    ctx.close()


_CACHE = {}


def _get_program():
    if "nc" not in _CACHE:
        _CACHE["nc"] = _build_program()
    return _CACHE["nc"]


def _prep_shared(fc_w, fc_b, ws):
    """Host layout prep for the replicated weights (shared by all cores)."""
    sh = {}
    sh["fcwT"] = np.ascontiguousarray(
        fc_w.T.reshape(2, 128, HID)).astype(BF)
    sh["fcb"] = np.ascontiguousarray(fc_b.reshape(KC0, 128, 1)).astype(F32)
    for l in range(3):
        w_ih, w_hh, b_ih, b_hh = ws[l]
        kcl = KC0 if l == 0 else KC
        sh[f"wihT{l}"] = np.ascontiguousarray(
            w_ih.T.reshape(kcl, 128, G3)).astype(BF)
        sh[f"whhT{l}"] = np.ascontiguousarray(
            w_hh.T.reshape(KC, 128, G3)).astype(BF)
    br = np.zeros((3, 128, MCH, BS), F32)
    bn = np.zeros((3, 128, FREE), F32)
    for l in range(3):
        _, _, b_ih, b_hh = ws[l]
        bi = b_ih.reshape(MCH, 128)
        bh = b_hh.reshape(MCH, 128)
        v = bi.copy()
        v[:2 * KC] += bh[:2 * KC]          # r,z gates absorb b_hh
        br[l] = v.T[:, :, None]
        bn[l] = np.repeat(bh[2 * KC:].T[:, :, None], BS, axis=2).reshape(128, FREE)
    sh["biasrep"] = br.reshape(3, 128, MCH * BS)
    sh["bhhn"] = bn
    sh["iota"] = np.broadcast_to(
        np.arange(T, dtype=F32)[None, :], (128, T)).copy()
    return sh


def kernel(z, seq_lens, chord_embedding, fc_w, fc_b,
           w_ih0, w_hh0, b_ih0, b_hh0,
           w_ih1, w_hh1, b_ih1, b_hh1,
           w_ih2, w_hh2, b_ih2, b_hh2):
    z = np.asarray(z, F32)
    chord = np.asarray(chord_embedding, F32)
    seq = np.asarray(seq_lens)
    ws = [(np.asarray(w_ih0, F32), np.asarray(w_hh0, F32),
           np.asarray(b_ih0, F32), np.asarray(b_hh0, F32)),
          (np.asarray(w_ih1, F32), np.asarray(w_hh1, F32),
           np.asarray(b_ih1, F32), np.asarray(b_hh1, F32)),
          (np.asarray(w_ih2, F32), np.asarray(w_hh2, F32),
           np.asarray(b_ih2, F32), np.asarray(b_hh2, F32))]

    in_maps = _make_in_maps(z, seq, chord, np.asarray(fc_w, F32),
                            np.asarray(fc_b, F32), ws)
    res = _execute(in_maps)
    return _assemble(res.results)


def _make_in_maps(z, seq, chord, fc_w, fc_b, ws):
    sh = _prep_shared(fc_w, fc_b, ws)
    in_maps = []
    for c in range(NCORES):
        bs = slice(c * BS, (c + 1) * BS)
        m = dict(sh)
        m["chordT"] = np.ascontiguousarray(
            (chord[bs].transpose(2, 1, 0) / 100.0)
            .reshape(KC0, 128, T, BS)).astype(BF)
        m["zT"] = np.ascontiguousarray(
            z[bs].T.reshape(2, 128, BS)).astype(BF)
        m["seqrep"] = np.broadcast_to(
            seq[bs].astype(F32)[None, :], (128, BS)).copy()
        in_maps.append(m)
    return in_maps


def _execute(in_maps, **kw):
    nc = _get_program()
    return bass_utils.run_bass_kernel_spmd(nc, in_maps, list(range(NCORES)), **kw)


def _assemble(results):
    out = np.empty((B, T, H), F32)
    for c in range(NCORES):
        outT = np.asarray(results[c]["outT"])       # [KC,128,T,BS]
        out[c * BS:(c + 1) * BS] = (
            outT.transpose(3, 2, 0, 1).reshape(BS, T, H))
    return out

